# revision 1
# baseline (speedup 1.0000x reference)
"""Trainium2 Bass kernel for nn_DenseBlockEnd (ragged masked residual-add + relu).

Op: out[g] = relu(features[g] + residuals[0,g] + residuals[1,g]) for rows < M_g,
    zeros for rows >= M_g  (M_g = mol_slice[g, 0]).

Strategy (8 NeuronCores, SPMD via run_bass_kernel_spmd):
- Shard the batch (B=256 graphs) across 8 cores, 32 graphs each, snake-draft
  balanced on total valid rows so per-core HBM traffic is equal.
- Ragged-aware device kernel: per graph, only the M_g valid rows are loaded
  (flat [128, M_g*8] f32 tiles so every DMA spans all 128 SBUF partitions at
  full port bandwidth), summed on DVE, relu'd on ACT, and stored. Rows >= M_g
  are never touched: ExternalOutput buffers are zero-initialized by the
  runtime (both the native and the PJRT/axon paths of run_bass_kernel_spmd),
  which materializes the masked zeros for free.
- Per-core schedules differ (ragged M values), so the single SPMD program
  branches on nc.partition_id() into 8 per-core instruction sequences.
"""

import sys

sys.path.insert(0, "/opt/trn_rl_repo")

from contextlib import ExitStack

import numpy as np

import bass_rust
import concourse.bass as bass
import concourse.mybir as mybir
from concourse.alu_op_type import AluOpType
import concourse.tile as tile
from concourse.bass_utils import run_bass_kernel_spmd
from concourse.tile import TileContext
from concourse.vector_clock import ScopedClock

B, A, F = 256, 128, 1024
N_CORES = 8
G_PER_CORE = B // N_CORES
GRAPH_ELEMS = A * F  # 131072 f32 per graph per stream


def _drain_and_barrier_split(self, tick_clock, wait_clock):
    # This container's walrus rejects instructions carrying more than one sem
    # wait ("Too many sync wait commands" at the kernel-tail Drain). Collect
    # the final waits on a probe instruction and emit them as single-wait
    # NOPs on the sync engine before a clean drain.
    probe = mybir.InstNoOp(
        name=self.nc.get_next_instruction_name(), engine=mybir.EngineType.SP
    )
    wait_clock.add_sem_waits(probe, ScopedClock({None: tick_clock.global_clock}))
    waits = list(probe.sync_info.on_wait) if probe.sync_info else []
    for w in waits:
        ins = self.nc.sync.nop(nofuse=True)
        si = ins.ins.sync_info
        if si is None:
            ins.ins.sync_info = mybir.SyncInfo(on_wait=[w], on_update=[])
        else:
            si.on_wait.append(w)
    self.nc.sync.drain()
    self.nc.all_engine_barrier()
    assert self.sems is not None
    popped = self.nc._tile_sem_poison_stack.pop()
    assert popped is self._sem_poison
    self.nc.clear_and_free_semaphores(list(self.sems.allocated().values()))
    if not getattr(self, "_skip_final_barrier", False):
        self.nc.all_engine_barrier()


tile.TileContext._drain_and_barrier = _drain_and_barrier_split

_orig_lower_ordered_insts = tile.TileContext._lower_ordered_insts


def _lower_with_wait_split(self, ordered):
    # Same walrus limitation as above, applied to every scheduled
    # instruction: hoist all but one sem wait onto single-wait NOPs emitted
    # just before the instruction on the same engine.
    for insts in ordered.values():
        if not any(
            i.sync_info is not None and len(i.sync_info.on_wait) > 1 for i in insts
        ):
            continue
        new_list = []
        for inst in insts:
            si = inst.sync_info
            if si is not None and len(si.on_wait) > 1:
                for w in si.on_wait[1:]:
                    new_list.append(
                        mybir.InstNoOp(
                            name=self.nc.get_next_instruction_name(),
                            engine=inst.engine,
                            sync_info=mybir.SyncInfo(on_wait=[w], on_update=[]),
                            bass_nofuse=True,
                        )
                    )
                si.on_wait = si.on_wait[:1]
            new_list.append(inst)
        insts[:] = new_list
    return _orig_lower_ordered_insts(self, ordered)


tile.TileContext._lower_ordered_insts = _lower_with_wait_split


def _assign_graphs(m: np.ndarray) -> list[list[int]]:
    """Snake-draft 256 graphs into 8 groups of 32, balancing sum(M)."""
    order = np.argsort(-m, kind="stable")
    groups: list[list[int]] = [[] for _ in range(N_CORES)]
    for rnd in range(G_PER_CORE):
        idxs = order[rnd * N_CORES : (rnd + 1) * N_CORES]
        seq = range(N_CORES) if rnd % 2 == 0 else range(N_CORES - 1, -1, -1)
        for c, g in zip(seq, idxs):
            groups[c].append(int(g))
    return groups


def _build_program(
    ms_per_core: tuple[tuple[int, ...], ...],
    bufs: int = 12,
    obufs: int | None = None,
    n_tail_hwdge: int = 4,
    swdge_queues: int = 1,
    hints: bool = False,
    n_sync_head: int = 0,
):
    obufs = bufs if obufs is None else obufs
    nc = bass.Bass(num_swdge_queues=swdge_queues)
    # f, r0, r1 packed host-side into one [3, G*A*F] input so each graph's
    # three valid regions load in ONE DMA (uniform stream stride).
    x_ext = nc.dram_tensor(
        "x", [3, G_PER_CORE * GRAPH_ELEMS], mybir.dt.float32, kind="ExternalInput"
    )
    o_ext = nc.dram_tensor(
        "o", [G_PER_CORE * GRAPH_ELEMS], mybir.dt.float32, kind="ExternalOutput"
    )

    def in_ap(g, w8):
        off = g * GRAPH_ELEMS
        # [128, 3, w8]: partition-major flat view of the graph's valid rows,
        # for all three streams at stride G*A*F.
        return x_ext[:, off : off + 128 * w8].rearrange("s (p w) -> p s w", p=128)

    def out_ap(g, w8):
        off = g * GRAPH_ELEMS
        return o_ext[off : off + 128 * w8].rearrange("(p w) -> p w", p=128)

    def load_reduce_relu(pool, opool, g, w8, n_sync_head=0):
        t = pool.tile([128, 3 * w8], mybir.dt.float32, tag="t")
        to = opool.tile([128, w8], mybir.dt.float32, tag="to")
        t3 = t[:].rearrange("p (s w) -> p s w", s=3)
        # SP skips the entry barrier, so the first few loads all go to it
        if g < n_sync_head:
            ld = nc.sync
        else:
            ld = nc.sync if g % 2 == 0 else nc.scalar
        ld.dma_start(out=t3, in_=in_ap(g, w8))
        # single 1-port DVE pass: sum the 3 streams (innermost axis,
        # stride w8) into the small out tile; frees the big tile early
        # and avoids 2-port DVE locks that stall SWDGE descriptor gen
        nc.vector.tensor_reduce(
            out=to[:],
            in_=t[:].rearrange("p (s w) -> p w s", s=3),
            axis=bass_rust.AxisListType.X,
            op=AluOpType.add,
        )
        nc.scalar.activation(
            out=to[:], in_=to[:], func=mybir.ActivationFunctionType.Relu
        )
        return to

    def core_body(pool, opool, ms):
        for g in range(len(ms)):
            m = ms[g]
            w8 = m * 8
            to = load_reduce_relu(pool, opool, g, w8, n_sync_head)
            if g >= len(ms) - n_tail_hwdge:
                # loads are finished by now: the HWDGE rings are idle, and
                # these late stores can't head-of-line-block any load
                st = nc.sync if g % 2 == 0 else nc.scalar
            else:
                st = nc.gpsimd
            st.dma_start(out=out_ap(g, w8), in_=to[:])

    with TileContext(nc) as tc:
        pid = nc.partition_id()
        with (
            tc.tile_pool(name="p", bufs=bufs) as pool,
            tc.tile_pool(name="po", bufs=obufs) as opool,
        ):
            if hints:
                # arm IRAM prefetch of this core's branch body: hint expr
                # lowers to 0 (LIKELY_TAKEN) only on the matching core
                for c in range(N_CORES - 1):
                    tc.mark_branch_hint_location(
                        f"corebr{c}", hint=pid - c, engines=mybir.ALL_ENGINES
                    )
            with ExitStack() as es:
                for c in range(N_CORES - 1):
                    cmp = tc.If(
                        pid == c,
                        preferred_fallthrough_block=False,
                        label=f"corebr{c}" if hints else None,
                    )
                    cm = cmp.__enter__()
                    core_body(pool, opool, ms_per_core[c])
                    cmp.__exit__(None, None, None)
                    es.enter_context(cm.Else())
                core_body(pool, opool, ms_per_core[N_CORES - 1])
    _exempt_sp_from_entry_barrier(nc)
    return nc


def _exempt_sp_from_entry_barrier(nc):
    """Let the SP engine skip the kernel-entry all-engine barrier.

    The preamble barrier only guards the Pool-engine const-AP memsets (which
    SP never reads) while absorbing ~4us of engine start skew. Removing SP's
    arrive+wait lets its first load DMAs start immediately. The barrier
    protocol is self-resetting, so only the entry barrier leader's counts
    change (4 -> 3).
    """
    f0 = nc.m.functions[0]
    bb0 = f0.blocks[0]
    exempt = (mybir.EngineType.SP,)
    pool = mybir.EngineType.Pool
    arrive_id = None
    evsems = []
    for ins in bb0.instructions:
        if ins.engine not in exempt or ins.sync_info is None:
            continue
        if ins.opcode == "Drain" and ins.sync_info.on_update:
            arrive_id = ins.sync_info.on_update[0].id
            ins.sync_info.on_update = []
            ins.sync_info.on_wait = []
        elif ins.opcode == "EventSemaphore" and arrive_id is not None:
            evsems.append(ins)
    if arrive_id is None or len(evsems) != len(exempt):
        return
    for ins in evsems:
        bb0.instructions.remove(ins)
    n = 4 - len(exempt)
    for ins in bb0.instructions:
        if ins.engine != pool or ins.opcode != "EventSemaphore" or ins.sync_info is None:
            continue
        si = ins.sync_info
        for w in si.on_wait:
            if w.id == arrive_id and w.wait_value == 4:
                w.wait_value = n
        for u in si.on_update:
            if u.update_value == 4:
                u.update_value = n


_PROGRAM_CACHE: dict = {}


def kernel(features, residuals, mol_slice):
    features = np.ascontiguousarray(np.asarray(features, dtype=np.float32))
    residuals = np.asarray(residuals, dtype=np.float32)
    mol_slice = np.asarray(mol_slice)
    m = mol_slice[:, 0].astype(np.int64)
    assert features.shape == (B, A, F) and residuals.shape == (2, B, A, F)

    groups = _assign_graphs(m)
    ms_per_core = tuple(tuple(int(m[g]) for g in groups[c]) for c in range(N_CORES))

    key = ms_per_core
    nc = _PROGRAM_CACHE.get(key)
    if nc is None:
        nc = _build_program(ms_per_core)
        _PROGRAM_CACHE[key] = nc

    in_maps = []
    for c in range(N_CORES):
        idx = np.array(groups[c], dtype=np.int64)
        x = np.empty((3, G_PER_CORE * GRAPH_ELEMS), dtype=np.float32)
        x[0] = features[idx].reshape(-1)
        x[1] = residuals[0][idx].reshape(-1)
        x[2] = residuals[1][idx].reshape(-1)
        in_maps.append({"x": x})

    res = run_bass_kernel_spmd(nc, in_maps, list(range(N_CORES)))

    out = np.zeros((B, A, F), dtype=np.float32)
    for c in range(N_CORES):
        core_out = res.results[c]["o"].reshape(G_PER_CORE, A, F)
        out[np.array(groups[c], dtype=np.int64)] = core_out
    return out

